# revision 68
# baseline (speedup 1.0000x reference)
# Multi-head causal attention (b=2, n=2048, dim=1024, 16 heads) on 8 TRN2
# NeuronCores. Sharding: core c -> batch c//4, head-group c%4 (4 heads x 64
# channels = 256). Host pre-transposes x and the weight slices so every
# device-side matmul contracts over the partition dimension.
#
# Device-side layout (per core):
#   xT   [1024, 2048] f32r  x[b].T (streamed in 512-col chunks)
#   QTz  [128, 4, 2048] bf16 per-head Q^T, off-head rows zeroed (so the
#                       K=128 contraction with a 2-head-packed K tile picks
#                       out one head); softmax scale folded into wqT on host
#   KT   [128, 2, 2048] f32r K^T, two heads packed per 128-partition block
#   Vp   [128, 16, 4, 65] bf16 V in [j, head, d+1]; col 64 = ones so the AV
#                       matmul also produces the softmax denominator
#   S^T  [j, i] psum tiles -> exp (ACT) -> es (f32r, so the AV lhsT
#        self-loads: no per-matmul Ldweights on the PE sequencer).
#   AV   contracts j as the PARTITION dim: psum [i, (ib,d+1)] per head, so
#        the denominator is per-partition and normalization is ONE fused
#        DVE tensor_tensor divide per (ic,h) (broadcast AP on the
#        denominator column) -> A [i, ch] bf16.  The psum tile is
#        pre-zeroed by a DVE memset and every AV matmul accumulates with
#        start=False (avoids the whole-bank pending-zero hazard of
#        interleaved start flags).
#   A -> AT via DMA XBAR transpose (bf16, zero engine cost) -> Wo (bf16)
#     -> bf16 partial out (host sums 4 partials per batch).
# Causal handling: S/exp/AV tiles trimmed at 128-col granularity; the one
# partially-valid 128-wide triangle per j-block is masked on the otherwise
# idle GPSIMD engine.
import os

if os.environ.get("JAX_PLATFORMS") == "cpu":
    # bass2jax must see the axon/neuron PJRT devices.
    del os.environ["JAX_PLATFORMS"]

from contextlib import ExitStack

import numpy as np

import concourse.bass as bass
import concourse.bacc as bacc
import concourse.mybir as mybir
import concourse.tile as tile
from concourse import bass_utils

F32 = mybir.dt.float32
F32R = mybir.dt.float32r
BF16 = mybir.dt.bfloat16
F8 = mybir.dt.float8e4
DR = mybir.MatmulPerfMode.DoubleRow
AF = mybir.ActivationFunctionType
ALU = mybir.AluOpType

P = 128
SEQ = 2048
DIM = 1024
CH = 256          # channels per core (4 heads x 64)
HD = 64           # head dim
NH = 4            # heads per core
KO = DIM // P     # 8 contraction chunks
NIC = SEQ // 512  # 4 i-chunks
NIB = SEQ // P    # 16 i-blocks
SCALE = float(HD) ** -0.5
# Projections run in residual-compensated fp8 (x~x8+dx8, w~w8+dw8; the
# dx8*dw8 cross term is dropped): ~0.15% error, 25% fewer PE cycles via
# DoubleRow.  Weights are scaled into fp8's normal range host-side; the net
# 8192x score inflation is undone in the exp scale, and V's 32x rides the
# denominator column (o2=32) through the softmax divide.
QF8 = 256.0 * SCALE
KF8 = 32.0
VF8 = 32.0
ESCALE = 1.0 / (256.0 * 32.0)
WO_DUE = int(os.environ.get("WO_DUE", "5"))
FINT_DUE = int(os.environ.get("FINT_DUE", "2"))
AVLAG = int(os.environ.get("AVLAG", "10"))
TAPER = int(os.environ.get("TAPER", "16"))


def build_nc():
    nc = bacc.Bacc("TRN2", target_bir_lowering=False, debug=False, num_devices=8)
    xT = nc.dram_tensor("xT", [DIM, SEQ], F8, kind="ExternalInput").ap()
    dxT = nc.dram_tensor("dxT", [DIM, SEQ], F8, kind="ExternalInput").ap()
    wqT = nc.dram_tensor("wqT", [DIM, 2, CH], F8, kind="ExternalInput").ap()
    wkT = nc.dram_tensor("wkT", [DIM, 2, CH], F8, kind="ExternalInput").ap()
    wvT = nc.dram_tensor("wvT", [DIM, 2, CH], F8, kind="ExternalInput").ap()
    woT = nc.dram_tensor("woT", [CH, DIM], BF16, kind="ExternalInput").ap()
    o2 = nc.dram_tensor("o2", [P, HD], BF16, kind="ExternalInput").ap()
    out = nc.dram_tensor("out", [SEQ, DIM], BF16, kind="ExternalOutput").ap()

    with ExitStack() as ctx:
        tc = ctx.enter_context(tile.TileContext(nc))
        per = ctx.enter_context(tc.tile_pool(name="persist", bufs=1))
        wpool = ctx.enter_context(tc.tile_pool(name="wts", bufs=1))
        xpool = ctx.enter_context(tc.tile_pool(name="xch", bufs=4))
        espool = ctx.enter_context(tc.tile_pool(name="es", bufs=14))
        apool = ctx.enter_context(tc.tile_pool(name="asb", bufs=4))
        opool = ctx.enter_context(tc.tile_pool(name="osb", bufs=8))
        dpool = ctx.enter_context(tc.tile_pool(name="dnm", bufs=3))
        psS = ctx.enter_context(tc.tile_pool(name="psS", bufs=2, space="PSUM"))
        psAV = ctx.enter_context(tc.tile_pool(name="psAV", bufs=2, space="PSUM"))
        psP = ctx.enter_context(tc.tile_pool(name="psP", bufs=2, space="PSUM"))

        QTz = per.tile([P, NH, SEQ], BF16)
        KT = per.tile([P, 2, SEQ], BF16)
        Vp = per.tile([P, NIB, NH, HD + 1], BF16)
        AT = per.tile([P, 2, SEQ], BF16)
        mfull = per.tile([P, P], F32)    # f32 staging for the mask build
        mask01 = per.tile([P, P], BF16)  # causal triangle: 1 where col >= row

        # weight tiles hold [w8, dw8] pairs in dim1
        wq_s = wpool.tile([P, KO, 2, CH], F8)
        wk_s = wpool.tile([P, KO, 2, CH], F8)
        wv_s = wpool.tile([P, KO, 2, CH], F8)
        wo_s = wpool.tile([P, 2, DIM], BF16)

        xch = {}
        dxch = {}

        def load_x(ic):
            t = xpool.tile([P, KO, 512], F8, tag="xch", name=f"xch{ic}")
            d = xpool.tile([P, KO, 512], F8, tag="dxch", name=f"dxch{ic}")
            nc.sync.dma_start(
                t[:], xT[:, ic * 512:(ic + 1) * 512].rearrange(
                    "(ko p) f -> p ko f", p=P)
            )
            nc.sync.dma_start(
                d[:], dxT[:, ic * 512:(ic + 1) * 512].rearrange(
                    "(ko p) f -> p ko f", p=P)
            )
            xch[ic] = t
            dxch[ic] = d

        # DMA order matters for startup latency: all prologue tensors are
        # small fp8 loads; get them in before the bulk x chunks.
        nc.sync.dma_start(wq_s[:], wqT.rearrange("(ko p) two c -> p ko two c", p=P))
        load_x(0)
        nc.sync.dma_start(wk_s[:], wkT.rearrange("(ko p) two c -> p ko two c", p=P))
        nc.sync.dma_start(wv_s[:], wvT.rearrange("(ko p) two c -> p ko two c", p=P))
        nc.sync.dma_start(Vp[:, :, :, HD], o2.rearrange("p (a b) -> p a b", a=NIB))
        load_x(1)
        nc.sync.dma_start(wo_s[:], woT.rearrange("(co p) f -> p co f", p=P))

        # scr memset FIRST on gpsimd so the PE warmup isn't stuck behind the
        # (slow) QTz zero-fills in the in-order Pool queue.
        scr = wpool.tile([P, 512], F32)
        nc.gpsimd.memset(scr[:], 0.0)

        # PE warmup: dummy fp32 matmuls keep the p-state ramp hot while the
        # first DMAs land; 2 are enough to cover the ~4us DMA window and not
        # head-of-line block the first projection.
        wps = psP.tile([P, 512], F32, tag="psP", name="warm")
        for _ in range(2):
            nc.tensor.matmul(
                wps[:], lhsT=scr[:, 0:P], rhs=scr[:, 0:512], start=True, stop=True
            )

        # causal triangle mask first (ic0's units are all diagonal and need
        # it immediately): mask01[p, f] = 1 where f >= p else 0
        nc.gpsimd.memset(mfull[:], 1.0)
        nc.gpsimd.affine_select(
            out=mfull[:],
            in_=mfull[:],
            compare_op=ALU.is_ge,
            fill=0.0,
            base=0,
            channel_multiplier=-1,
            pattern=[[1, P]],
        )
        nc.vector.tensor_copy(mask01[:], mfull[:])

        # Off-head rows of QTz zeroed on the idle gpsimd engine during the
        # DMA phase (bf16 memset is fine; only f32r can't be memset).
        for hh in range(NH):
            off0 = HD if hh % 2 == 0 else 0
            nc.gpsimd.memset(QTz[off0:off0 + HD, hh, :], 0.0)

        def proj_mm_unit(ic, co, w_s, name):
            # residual fp8 DoubleRow: x8*w8 + x8*dw8 + dx8*w8, each chain 4
            # double-row passes over ko pairs, all into one psum.
            ps = psP.tile([P, 512], F32, tag="psP", name=f"p{name}{ic}{co}")
            chains = [
                (xch[ic], 0), (xch[ic], 1), (dxch[ic], 0),
            ]
            for ci, (xs, wi) in enumerate(chains):
                for kp in range(KO // 2):
                    nc.tensor.matmul(
                        ps[:],
                        lhsT=w_s[:, 2 * kp:2 * kp + 2, wi, co * P:(co + 1) * P],
                        rhs=xs[:, 2 * kp:2 * kp + 2, :],
                        start=(ci == 0 and kp == 0),
                        stop=(ci == 2 and kp == KO // 2 - 1),
                        perf_mode=DR,
                    )
            return ps

        def _cp(eng):
            return nc.scalar.copy if eng == 'act' else nc.vector.tensor_copy

        def proj_q_evac(ic, co, ps, eng=None):
            cp = _cp(eng)
            i0 = ic * 512
            cp(QTz[0:HD, 2 * co, i0:i0 + 512], ps[0:HD, :])
            cp(QTz[HD:P, 2 * co + 1, i0:i0 + 512], ps[HD:P, :])

        def proj_k_evac(ic, co, ps, eng=None):
            i0 = ic * 512
            _cp(eng)(KT[:, co, i0:i0 + 512], ps[:])

        def proj_v_mm(ic, g):
            ps = psP.tile([P, 512], F32, tag="psP", name=f"pv{ic}{g}")
            for u in range(2):
                sl = slice((2 * g + u) * P, (2 * g + u + 1) * P)
                chains = [
                    (xch[ic], 0), (xch[ic], 1), (dxch[ic], 0),
                ]
                for ci, (xs, wi) in enumerate(chains):
                    for kp in range(KO // 2):
                        nc.tensor.matmul(
                            ps[:, u * 256:(u + 1) * 256],
                            lhsT=xs[:, 2 * kp:2 * kp + 2, sl],
                            rhs=wv_s[:, 2 * kp:2 * kp + 2, wi, :],
                            start=(ci == 0 and kp == 0),
                            stop=(ci == 2 and kp == KO // 2 - 1),
                            perf_mode=DR,
                        )
            return ps

        def proj_v_evac(ic, g, ps, eng=None):
            _cp(eng)(
                Vp[:, 4 * ic + 2 * g:4 * ic + 2 * g + 2, :, 0:HD],
                ps[:].rearrange("p (j h d) -> p j h d", j=2, h=NH),
            )

        def proj_fillers(ic):
            st = {}
            units = []
            for co in range(2):
                units.append(lambda co=co: st.__setitem__(
                    ('q', co), proj_mm_unit(ic, co, wq_s, 'q')))
                units.append(lambda co=co: proj_q_evac(ic, co, st.pop(('q', co))))
            for co in range(2):
                units.append(lambda co=co: st.__setitem__(
                    ('k', co), proj_mm_unit(ic, co, wk_s, 'k')))
                units.append(lambda co=co: proj_k_evac(ic, co, st.pop(('k', co))))
            for g in range(2):
                units.append(lambda g=g: st.__setitem__(('v', g), proj_v_mm(ic, g)))
                units.append(lambda g=g: proj_v_evac(ic, g, st.pop(('v', g))))
            return units

        ob_tiles = {}
        store_fq = []  # (due_pos, fn): stores are emitted a few positions
                       # after their ob copy so they never head-of-line block
                       # the transposes sharing the SP DMA queue

        def wo_unit(ib, fc, pos=None):
            def emit():
                ps2 = psP.tile([P, 512], F32, tag="psP", name=f"po{ib}{fc}")
                for cc in range(2):
                    nc.tensor.matmul(
                        ps2[:],
                        lhsT=AT[:, cc, ib * P:(ib + 1) * P],
                        rhs=wo_s[:, cc, fc * 512:(fc + 1) * 512],
                        start=(cc == 0),
                        stop=(cc == 1),
                    )
                if fc == 0:
                    ob_tiles[ib] = opool.tile(
                        [P, 2, 512], BF16, tag="ob", name=f"ob{ib}"
                    )
                ob = ob_tiles[ib]
                nc.vector.tensor_copy(ob[:, fc, :], ps2[:])
                if fc == 1:
                    def store(ib=ib):
                        nc.sync.dma_start(
                            out[ib * P:(ib + 1) * P, :],
                            ob_tiles.pop(ib)[:].rearrange("p a b -> p (a b)"),
                        )
                    store_fq.append(store)
            return emit

        def wo_units(ic):
            return [wo_unit(4 * ic + ib, fc) for ib in range(4) for fc in range(2)]

        def wo_unit_wide(ib):
            # tail-only: the S-stream is done, so the psS pool is free; do
            # both fc halves in one [128,1024] psum + one copy + one store.
            ps2 = psS.tile([P, 1024], F32, tag="psS", name=f"pow{ib}")
            for fc in range(2):
                for cc in range(2):
                    nc.tensor.matmul(
                        ps2[:, fc * 512:(fc + 1) * 512],
                        lhsT=AT[:, cc, ib * P:(ib + 1) * P],
                        rhs=wo_s[:, cc, fc * 512:(fc + 1) * 512],
                        start=(cc == 0),
                        stop=(cc == 1),
                    )
            ob = opool.tile([P, 1024], BF16, tag="obw", name=f"obw{ib}")
            if ib % 2 == 0:
                nc.scalar.copy(ob[:], ps2[:])   # ACT is idle in the tail
            else:
                nc.vector.tensor_copy(ob[:], ps2[:])
            nc.sync.dma_start(out[ib * P:(ib + 1) * P, :], ob[:])

        # Minimal prologue: the first S units need only the co=0 Q/K slices;
        # with the deep AV lag even V(0) can stream in as a filler.
        proj_q_evac(0, 0, proj_mm_unit(0, 0, wq_s, 'q'), eng='act')
        proj_k_evac(0, 0, proj_mm_unit(0, 0, wk_s, 'k'), eng='act')
        load_x(2)
        load_x(3)

        # ---- one continuous S -> exp -> AV pipeline across every (ic, h) ----
        S_units = []
        for ic in range(NIC):
            for h in range(NH):
                for t in range(2 * ic + 2):
                    S_units.append((ic, h, t))

        es_tiles = {}
        pAV_tiles = {}   # h -> psum tile [P, NH_ib=4, HD+1], per current ic
        A_tiles = {}     # ic -> sbuf tile [P, 4, CH]
        proj_fq = []     # projection units: must drain before the next ic
        wo_fq = []       # output-projection units: emit whenever
        delayed = []     # (due_pos, fn)

        def emit_S(ic, h, t):
            co = h // 2
            pS = psS.tile([P, 1024], F32, tag="psS", name=f"pS{ic}{h}{t}")
            for u in range(2):
                jb = 2 * t + u
                r = jb - 4 * ic
                off = 0 if r < 0 else P * r
                nc.tensor.matmul(
                    pS[:, u * 512 + off:(u + 1) * 512],
                    lhsT=KT[:, co, jb * P:(jb + 1) * P],
                    rhs=QTz[:, h, ic * 512 + off:(ic + 1) * 512],
                    start=True,
                    stop=True,
                )
            es = espool.tile([P, 1024], BF16, tag="es", name=f"es{ic}{h}{t}")
            if t >= 2 * ic:  # diagonal unit
                for u in range(2):
                    r = 2 * t + u - 4 * ic
                    off = P * r
                    nc.scalar.activation(
                        es[:, u * 512 + off:(u + 1) * 512],
                        pS[:, u * 512 + off:(u + 1) * 512],
                        AF.Exp,
                        scale=ESCALE,
                    )
                    # only the 128-wide diagonal triangle needs masking
                    nc.gpsimd.tensor_mul(
                        es[:, u * 512 + off:u * 512 + off + P],
                        es[:, u * 512 + off:u * 512 + off + P],
                        mask01[:],
                    )
            else:
                nc.scalar.activation(es[:], pS[:], AF.Exp, scale=ESCALE)
            es_tiles[(ic, h, t)] = es

        def emit_AV(pos, ic, h, t):
            es = es_tiles.pop((ic, h, t))
            if t == 0:
                pAV = pAV_tiles[h] = psAV.tile(
                    [P, 4, HD + 1], F32, tag="psAV", name=f"pAV{ic}{h}"
                )
                nc.vector.memset(pAV[:], 0.0)  # accumulate base; no start flags
            else:
                pAV = pAV_tiles[h]
            for u in range(2):
                jb = 2 * t + u
                r = jb - 4 * ic
                for ib in range(4):
                    if r > ib:
                        continue  # block fully above the diagonal
                    nc.tensor.matmul(
                        pAV[:, ib, :],
                        lhsT=es[:, u * 512 + ib * P:u * 512 + (ib + 1) * P],
                        rhs=Vp[:, jb, h, :],
                        start=False,
                        stop=(jb == 4 * ic + ib),
                        skip_group_check=True,
                    )
            if t == 2 * ic + 1:  # head finished: normalize + evacuate
                def fin(ic=ic, h=h, pos=pos):
                    if ic not in A_tiles:
                        A_tiles[ic] = apool.tile(
                            [P, 4, CH], BF16, tag="asb", name=f"A{ic}"
                        )
                    A = A_tiles[ic]
                    pAV = pAV_tiles.pop(h)
                    num = pAV[:, :, 0:HD]
                    # DVE can read only one non-scalar PSUM input and free
                    # broadcast (stride-0) APs don't codegen: stage + invert
                    # the denominator column in SBUF, then one per-partition
                    # scalar multiply per i-block.
                    dnm = dpool.tile([P, 4], F32, tag="dnm", name=f"d{ic}{h}")
                    nc.vector.tensor_copy(dnm[:], pAV[:, :, HD:HD + 1])
                    with nc.allow_low_precision(reason="softmax denom recip"):
                        nc.vector.reciprocal(dnm[:], dnm[:])
                    for ib in range(4):
                        nc.vector.tensor_scalar(
                            out=A[:, ib, h * HD:(h + 1) * HD],
                            in0=num[:, ib],
                            scalar1=dnm[:, ib:ib + 1],
                            scalar2=None,
                            op0=ALU.mult,
                        )
                    if h == NH - 1:
                        def finT(ic=ic):
                            A = A_tiles.pop(ic)
                            for ib in range(4):
                                for cc in range(2):
                                    nc.sync.dma_start_transpose(
                                        AT[:, cc, (4 * ic + ib) * P:(4 * ic + ib + 1) * P],
                                        A[:, ib, cc * P:(cc + 1) * P],
                                    )
                            wo_fq.extend((pos + WO_DUE, u) for u in wo_units(ic))
                        delayed.append((pos + FINT_DUE, finT))
                delayed.append((pos + 1, fin))

        def run_due(pos):
            while delayed and delayed[0][0] <= pos:
                delayed.pop(0)[1]()

        # rest of ic0's projections as immediate fillers; Q/K co=1 before
        # h2's S units (position 4), V before the first AV (position ~10).
        st0 = {}
        proj_fq.extend([
            lambda: st0.__setitem__('q', proj_mm_unit(0, 1, wq_s, 'q')),
            lambda: proj_q_evac(0, 1, st0.pop('q'), eng='act'),
            lambda: st0.__setitem__('k', proj_mm_unit(0, 1, wk_s, 'k')),
            lambda: proj_k_evac(0, 1, st0.pop('k'), eng='act'),
            lambda: st0.__setitem__('v', proj_v_mm(0, 0)),
            lambda: proj_v_evac(0, 0, st0.pop('v'), eng='act'),
            lambda: st0.__setitem__('u', proj_v_mm(0, 1)),
            lambda: proj_v_evac(0, 1, st0.pop('u'), eng='act'),
        ])

        npos = len(S_units)
        next_av = [0]
        for pos, (ic, h, t) in enumerate(S_units):
            if t == 0 and h == 0:
                while proj_fq:  # safety: next ic's inputs must exist by now
                    proj_fq.pop(0)()
                if ic + 1 < NIC:
                    proj_fq.extend(proj_fillers(ic + 1))
            if ic == 0:  # drain ic0 leftovers aggressively
                for _ in range(2):
                    if proj_fq:
                        proj_fq.pop(0)()
            emit_S(ic, h, t)
            run_due(pos)
            # AV lag ramps down across ic2 so ic3 is entered with no
            # deferred backlog (the tail then starts right after the last
            # exp instead of draining ~10 units of AV first).
            def lagf(u):
                if u < 48:
                    return AVLAG
                if u < 64:
                    return max(2, AVLAG - (u - 48) // 2)
                return 2
            for _ in range(3):
                u = next_av[0]
                if u < npos and u <= pos - 2 and u + lagf(u) <= pos:
                    next_av[0] += 1
                    emit_AV(pos, *S_units[u])
            if t == 2 * ic + 1:  # head's S-units done -> slip in fillers
                for _ in range(4):
                    if proj_fq:
                        proj_fq.pop(0)()
                for _ in range(2):
                    if wo_fq and wo_fq[0][0] <= pos:
                        wo_fq.pop(0)[1]()
                while len(store_fq) > 2:  # keep <=2 pending; emit the rest
                    store_fq.pop(0)()
        k = 0
        while next_av[0] < npos:
            u = next_av[0]
            next_av[0] += 1
            emit_AV(npos + k, *S_units[u])
            k += 1
        run_due(npos + AVLAG)
        run_due(npos + AVLAG + 20)
        while wo_fq:
            wo_fq.pop(0)[1]()
        while store_fq:
            store_fq.pop(0)()
        for ib in range(4):
            wo_unit_wide(12 + ib)

    nc.compile()
    return nc


_NC = None


def get_nc():
    global _NC
    if _NC is None:
        _NC = build_nc()
    return _NC


def make_in_maps(x, Wq, Wk, Wv, Wo):
    bf16 = mybir.dt.np(BF16)
    f8 = mybir.dt.np(F8)

    def f8pair(w):
        # [w8, dw8] stacked on a new axis 1: w ~ w8 + dw8 (dw8 lives in
        # fp8 subnormals; the quantization residue is ~0.1%)
        w8 = w.astype(f8)
        dw8 = (w - w8.astype(np.float32)).astype(f8)
        return np.stack([w8, dw8], axis=1)

    x = np.ascontiguousarray(np.asarray(x, dtype=np.float32))
    Wq = np.asarray(Wq, dtype=np.float32) * QF8
    Wk = np.asarray(Wk, dtype=np.float32) * KF8
    Wv = np.asarray(Wv, dtype=np.float32) * VF8
    Wo = np.asarray(Wo, dtype=np.float32)
    in_maps = []
    for c in range(8):
        b, g = divmod(c, 4)
        hs = g * CH
        xTb = np.ascontiguousarray(x[b].T)
        x8 = xTb.astype(f8)
        dx8 = (xTb - x8.astype(np.float32)).astype(f8)
        in_maps.append(
            {
                "xT": x8,
                "dxT": dx8,
                "wqT": f8pair(np.ascontiguousarray(Wq[hs:hs + CH, :].T)),
                "wkT": f8pair(np.ascontiguousarray(Wk[hs:hs + CH, :].T)),
                "wvT": f8pair(np.ascontiguousarray(Wv[hs:hs + CH, :].T)),
                "woT": np.ascontiguousarray(Wo[:, hs:hs + CH].T).astype(bf16),
                "o2": np.full((P, HD), VF8, dtype=bf16),
            }
        )
    return in_maps


LAST_RESULTS = None
_WARMED = False


def kernel(x, Wq, Wk, Wv, Wo, trace=False):
    global LAST_RESULTS, _WARMED
    nc = get_nc()
    in_maps = make_in_maps(x, Wq, Wk, Wv, Wo)
    if not _WARMED:
        # Discarded warm-up execution: the very first run on cold SBUF is
        # occasionally perturbed; steady-state runs are deterministic.
        bass_utils.run_bass_kernel_spmd(
            nc, in_maps, core_ids=list(range(8)), trace=False
        )
        _WARMED = True
    res = bass_utils.run_bass_kernel_spmd(
        nc, in_maps, core_ids=list(range(8)), trace=trace
    )
    LAST_RESULTS = res
    partials = [np.asarray(r["out"], dtype=np.float32) for r in res.results]
    out0 = partials[0] + partials[1] + partials[2] + partials[3]
    out1 = partials[4] + partials[5] + partials[6] + partials[7]
    return np.stack([out0, out1]).astype(np.float32)
